# revision 1
# baseline (speedup 1.0000x reference)
"""Trainium2 Bass kernel for nn_L2PppMaskAttn (topk_masking).

Math reformulation of the reference:
  - a_k = sum(l2norm(K[idx]) * l2norm(A[idx])) depends only on (layer, prompt):
    precompute s[l,p] = <K_hat[l,p], A_hat[l,p]> once per layer.
  - top-5 ranking over prompts is invariant to q normalization (positive
    per-row scale), so scores u[b,p] = <x[b,l], K_hat[l,p]> suffice.
  - out[l,b] = sum_{p in top5} s[l,p] * P[l,p] = (mask_row .* s) @ P_flat[l],
    a dense [B,100] @ [100, 6144] matmul per layer (topk -> masking).

Sharding: data-parallel over batch, 8 cores x 128 rows; K/A/P replicated.
"""

import sys

sys.path.insert(0, "/opt/trn_rl_repo")

import numpy as np

B, L, P_N, LP, D = 1024, 12, 100, 8, 768
N_CORES = 8
BS = B // N_CORES  # 128 batch rows per core
NF = LP * D  # 6144 flattened output features per layer
TOP_K = 5
NEG_BIG = -1.0e30

_CACHE = {}


def _build_nc():
    if "nc" in _CACHE:
        return _CACHE["nc"]

    from contextlib import ExitStack

    import concourse.bass as bass
    import concourse.bacc as bacc
    import concourse.mybir as mybir
    from concourse import masks
    from concourse.tile import TileContext

    f32 = mybir.dt.float32
    f32r = mybir.dt.float32r
    AX = mybir.AxisListType
    OP = mybir.AluOpType
    AF = mybir.ActivationFunctionType

    nc = bacc.Bacc(
        "TRN2",
        target_bir_lowering=False,
        debug=False,
        num_devices=N_CORES,
    )

    x_d = nc.declare_dram_parameter("x", [BS, L * D], f32, isOutput=False)
    k_d = nc.declare_dram_parameter("k", [L, P_N, D], f32, isOutput=False)
    a_d = nc.declare_dram_parameter("a", [L, P_N, D], f32, isOutput=False)
    p_d = nc.declare_dram_parameter("p", [L, P_N, NF], f32, isOutput=False)
    o_d = nc.declare_dram_parameter("o", [L, BS, NF], f32, isOutput=True)

    with TileContext(nc) as tc, ExitStack() as ctx:
        pool = lambda name, bufs, **kw: ctx.enter_context(
            tc.tile_pool(name=name, bufs=bufs, **kw)
        )
        const = pool("const", 1)
        xp = pool("xp", 2)
        kap = pool("kap", 2)
        scrp = pool("scrp", 2)
        nrm = pool("nrm", 2)
        nktp = pool("nktp", 2)
        xtp = pool("xtp", 2)
        ppool = pool("pp", 2)
        obuf = pool("ob", 2)
        small = pool("small", 3)
        rowp = pool("rowp", 2)
        wtp = pool("wtp", 2)
        ps_t = pool("ps_t", 2, space="PSUM")
        ps_c = pool("ps_c", 2, space="PSUM")
        ps_o = pool("ps_o", 4, space="PSUM")

        ident = const.tile([128, 128], f32)
        masks.make_identity(nc, ident[:])

        x_dv = x_d[:].rearrange("b (l d) -> b l d", l=L)

        for l in range(L):
            # ---- load pools for this layer ----
            x_sb = xp.tile([BS, D], f32)
            nc.sync.dma_start(x_sb[:], x_dv[:, l])
            ka = kap.tile([P_N, D], f32, tag="ka")
            nc.sync.dma_start(ka[:], k_d[l])
            aa = kap.tile([P_N, D], f32, tag="aa")
            nc.sync.dma_start(aa[:], a_d[l])
            # f32r (TF32-class) pool operand: rounded during the DMA cast,
            # runs the output matmul at 1 cycle/row instead of fp32's 4.
            p_sb = ppool.tile([P_N, NF], f32r)
            nc.gpsimd.dma_start(p_sb[:], p_d[l])

            # ---- l2 norms of K and A rows: rsqrt(sum(sq)) w/ Newton polish ----
            rs = []
            for src in (ka, aa):
                scr = scrp.tile([P_N, D], f32, tag="scr")
                ss = small.tile([P_N, 1], f32, tag="ss")
                nc.scalar.activation(scr[:], src[:], AF.Square, accum_out=ss[:])
                sq = small.tile([P_N, 1], f32, tag="sq")
                nc.scalar.activation(sq[:], ss[:], AF.Sqrt)
                y0 = small.tile([P_N, 1], f32, tag="y0")
                nc.vector.reciprocal(y0[:], sq[:])
                # one Newton step: y = y0 * (1.5 - 0.5 * ss * y0^2)
                t1 = small.tile([P_N, 1], f32, tag="t1")
                nc.vector.tensor_tensor(t1[:], y0[:], y0[:], op=OP.mult)
                nc.vector.tensor_tensor(t1[:], t1[:], ss[:], op=OP.mult)
                nc.vector.tensor_scalar(t1[:], t1[:], -0.5, 1.5, OP.mult, OP.add)
                y1 = small.tile([P_N, 1], f32, tag="y1")
                nc.vector.tensor_tensor(y1[:], t1[:], y0[:], op=OP.mult)
                rs.append(y1)

            nk = nrm.tile([P_N, D], f32, tag="nk")
            nc.vector.tensor_scalar_mul(nk[:], ka[:], rs[0][:])
            na = nrm.tile([P_N, D], f32, tag="na")
            nc.vector.tensor_scalar_mul(na[:], aa[:], rs[1][:])

            # s[p] = <nk_p, na_p>
            scr2 = scrp.tile([P_N, D], f32, tag="scr")
            s_t = small.tile([P_N, 1], f32, tag="s_t")
            nc.vector.tensor_tensor(scr2[:], nk[:], na[:], op=OP.mult)
            nc.vector.reduce_sum(s_t[:], scr2[:], axis=AX.X)

            # ---- transpose nk -> [768(=6x128), 100] and x[:, l] -> [768, 128] ----
            nkt = nktp.tile([128, 6 * P_N], f32)
            for j in range(6):
                pt = ps_t.tile([128, P_N], f32, tag="tp")
                nc.tensor.transpose(
                    pt[:], nk[:, j * 128 : (j + 1) * 128], ident[:P_N, :P_N]
                )
                nc.scalar.copy(nkt[:, j * P_N : (j + 1) * P_N], pt[:])
            xt = xtp.tile([128, D], f32)
            for j in range(6):
                pt = ps_t.tile([128, 128], f32, tag="tp")
                nc.tensor.transpose(
                    pt[:], x_sb[:, j * 128 : (j + 1) * 128], ident[:]
                )
                nc.scalar.copy(xt[:, j * 128 : (j + 1) * 128], pt[:])

            # ---- scores u = x_l @ nk.T : psum [128b, 100p] ----
            pc = ps_c.tile([BS, P_N], f32)
            for j in range(6):
                nc.tensor.matmul(
                    pc[:],
                    xt[:, j * 128 : (j + 1) * 128],
                    nkt[:, j * P_N : (j + 1) * P_N],
                    start=(j == 0),
                    stop=(j == 5),
                )
            cos = rowp.tile([BS, P_N], f32, tag="cos")
            nc.scalar.copy(cos[:], pc[:])
            work = rowp.tile([BS, P_N], f32, tag="work")
            nc.vector.tensor_copy(work[:], cos[:])

            # ---- iterative top-5: find 5th max per row ----
            mm = small.tile([BS, TOP_K], f32, tag="mm")
            pen = rowp.tile([BS, P_N], f32, tag="pen")
            for it in range(TOP_K):
                nc.vector.reduce_max(mm[:, it : it + 1], work[:], axis=AX.X)
                if it < TOP_K - 1:
                    nc.vector.tensor_scalar(
                        pen[:], work[:], mm[:, it : it + 1], NEG_BIG, OP.is_ge, OP.mult
                    )
                    nc.vector.tensor_tensor(work[:], work[:], pen[:], op=OP.add)

            # mask = (u >= t5) in {0,1}
            mask = rowp.tile([BS, P_N], f32, tag="mask")
            nc.vector.tensor_scalar(
                mask[:], cos[:], mm[:, TOP_K - 1 : TOP_K], None, OP.is_ge
            )

            # W^T = mask^T * s  -> [100, 128]
            mt = ps_t.tile([P_N, 128], f32, tag="tp")
            nc.tensor.transpose(mt[:], mask[:], ident[:])
            wt = wtp.tile([P_N, BS], f32r)
            nc.vector.tensor_scalar_mul(wt[:], mt[:], s_t[:])

            # ---- out[l] = W @ P_flat : 12 x [128, 512] matmuls ----
            ob = obuf.tile([BS, NF], f32)
            for n in range(12):
                po = ps_o.tile([BS, 512], f32)
                nc.tensor.matmul(
                    po[:], wt[:], p_sb[:, n * 512 : (n + 1) * 512], start=True, stop=True
                )
                if n % 2 == 0:
                    nc.scalar.copy(ob[:, n * 512 : (n + 1) * 512], po[:])
                else:
                    nc.vector.tensor_copy(ob[:, n * 512 : (n + 1) * 512], po[:])
            nc.sync.dma_start(o_d[l], ob[:])

    nc.compile()
    _CACHE["nc"] = nc
    return nc


def _run(x_query, K_all, A_all, P_all, trace=False, tmpdir=None):
    from concourse.bass_utils import run_bass_kernel_spmd

    x = np.ascontiguousarray(np.asarray(x_query, dtype=np.float32)).reshape(B, L * D)
    k = np.ascontiguousarray(np.asarray(K_all, dtype=np.float32))
    a = np.ascontiguousarray(np.asarray(A_all, dtype=np.float32))
    p = np.ascontiguousarray(np.asarray(P_all, dtype=np.float32)).reshape(L, P_N, NF)

    nc = _build_nc()
    in_maps = [
        {"x": x[c * BS : (c + 1) * BS], "k": k, "a": a, "p": p} for c in range(N_CORES)
    ]
    br = run_bass_kernel_spmd(
        nc, in_maps, list(range(N_CORES)), trace=trace, tmpdir=tmpdir
    )
    out = np.stack([r["o"] for r in br.results], axis=0)  # [8, L, BS, NF]
    out = out.transpose(1, 0, 2, 3).reshape(L, B, LP, D)
    return out, br


def kernel(x_query, K_all, A_all, P_all):
    out, _ = _run(x_query, K_all, A_all, P_all)
    return out



# revision 9
# speedup vs baseline: 1.5606x; 1.5606x over previous
"""Trainium2 Bass kernel for nn_L2PppMaskAttn (topk_masking).

Math reformulation of the reference:
  - top-5 ranking over prompts is invariant to q normalization, so scores
    u[b,p] = <x[b,l], K_hat[l,p]> suffice; mask = (u >= 5th_max(u)).
  - a_k depends only on (layer, prompt): s[l,p] = <K_hat[l,p], A_hat[l,p]>.
  - out[l,b] = (mask_row .* s) @ P_flat[l]: a [128,100] @ [100,6144] matmul.

Numerical contract: everything feeding the top-5 SELECTION replicates the
known-good op sequence bit-for-bit (scalar Square+accum for ||K||^2, sqrt,
reciprocal + one Newton step, f32 elementwise K*rinv products, PE f32
matmuls in the same 6x128-chunk accumulation order).  The tightest 5th/6th
score gap in this input set is ~1e-6, and a single flipped selection costs
~0.2 rel error, so this path must not be re-ordered.  The output-scale path
(s, P matmul, store) runs in bf16: ~5e-3 worst-case vs the 2e-2 gate.

Layouts are host-packed so the device does no transposes of x or K:
  x^T   [128dd, (l,j,b)]   f32   4.7 MB   (j = 128-col chunk of D)
  K^T   [128dd, (l,j,p)]   f32   3.7 MB   (feeds scores)
  K     [100p,  (l,d)]     f32   3.7 MB   (feeds ||K||^2, baseline-exact)
  A^T   [128dd, (l,j,p)]   bf16  1.8 MB   (feeds s via PE grams)
  P     [l][100p, 6144]    bf16 14.8 MB
  out   [l][128b, 6144]    bf16 18.9 MB
Total ~47.6 MB HBM traffic per core vs ~79 MB for the f32 baseline.

Sharding: data-parallel over batch, 8 cores x 128 rows; K/A/P replicated.
"""

import sys

sys.path.insert(0, "/opt/trn_rl_repo")

import numpy as np

B, L, P_N, LP, D = 1024, 12, 100, 8, 768
N_CORES = 8
BS = B // N_CORES  # 128 batch rows per core
NF = LP * D  # 6144 flattened output features per layer
NCH = D // 128  # 6 contraction chunks
TOP_K = 5

_CACHE = {}


def _build_nc():
    if "nc" in _CACHE:
        return _CACHE["nc"]

    from contextlib import ExitStack

    import concourse.bass as bass
    import concourse.bacc as bacc
    import concourse.mybir as mybir
    from concourse import masks
    from concourse.tile import TileContext

    f32 = mybir.dt.float32
    bf16 = mybir.dt.bfloat16
    AX = mybir.AxisListType
    OP = mybir.AluOpType
    AF = mybir.ActivationFunctionType

    nc = bacc.Bacc(
        "TRN2",
        target_bir_lowering=False,
        debug=False,
        num_devices=N_CORES,
    )

    xt_d = nc.declare_dram_parameter("x", [128, L * D], f32, isOutput=False)
    kt_d = nc.declare_dram_parameter("kt", [128, L * NCH * P_N], f32, isOutput=False)
    at_d = nc.declare_dram_parameter("at", [128, L * NCH * P_N], bf16, isOutput=False)
    kn_d = nc.declare_dram_parameter("kn", [P_N, L * D], f32, isOutput=False)
    p_d = nc.declare_dram_parameter("p", [L, P_N, NF], bf16, isOutput=False)
    o_d = nc.declare_dram_parameter("o", [L, BS, NF], bf16, isOutput=True)

    with TileContext(nc) as tc, ExitStack() as ctx:
        pool = lambda name, bufs, **kw: ctx.enter_context(
            tc.tile_pool(name=name, bufs=bufs, **kw)
        )
        const = pool("const", 1)
        perm = pool("perm", 1)
        ppool = pool("pp", 2)
        nktp = pool("nktp", 2)
        kbp = pool("kbp", 2)
        scrp = pool("scrp", 2)
        rowp = pool("rowp", 2)
        small = pool("small", 2)
        wtp = pool("wtp", 2)
        obuf = pool("ob", 2)
        # Every PE-written PSUM tile gets its own bank: concurrent PE-write +
        # DVE/ScalarE-read of the SAME bank is a hardware fatal, and the tile
        # scheduler only serializes element-overlapping accesses.
        ps_pc = pool("ps_pc", 1, space="PSUM")
        ps_rb = pool("ps_rb", 1, space="PSUM")
        ps_ka = pool("ps_ka", 1, space="PSUM")
        ps_aa = pool("ps_aa", 1, space="PSUM")
        ps_mt = pool("ps_mt", 1, space="PSUM")
        ps_o = pool("ps_o", 3, space="PSUM")

        ident = const.tile([128, 128], f32, tag="ident")
        masks.make_identity(nc, ident[:])
        ones_col = const.tile([100, 128], f32, tag="ones")
        nc.vector.memset(ones_col[:], 1.0)

        xt_all = perm.tile([128, L * D], f32, tag="xt")
        kt_all = perm.tile([128, L * NCH * P_N], f32, tag="kt")
        at_all = perm.tile([128, L * NCH * P_N], bf16, tag="at")
        kn_all = perm.tile([P_N, L * D], f32, tag="kn")

        # Layer-0 slices land first (~1 MB) so layer-0 compute starts a few
        # microseconds in; the remainders stream behind on the same ring.
        C = NCH * P_N
        nc.sync.dma_start(kn_all[:, :D], kn_d[:, :D])
        nc.sync.dma_start(kt_all[:, :C], kt_d[:, :C])
        nc.sync.dma_start(xt_all[:, :D], xt_d[:, :D])
        nc.sync.dma_start(kn_all[:, D:], kn_d[:, D:])
        nc.sync.dma_start(kt_all[:, C:], kt_d[:, C:])
        nc.sync.dma_start(xt_all[:, D:], xt_d[:, D:])
        nc.sync.dma_start(at_all[:], at_d[:])

        for l in range(L):
            # P streams on the SWDGE ring so it never queues behind the
            # bulk loads (sync ring) or the output stores (scalar ring).
            p_sb = ppool.tile([P_N, NF], bf16, tag="p")
            nc.gpsimd.dma_start(p_sb[:], p_d[l])

            # ---- rinv = 1/||K_p|| (selection-critical, baseline op order) ----
            scr = scrp.tile([P_N, D], f32, tag="scr")
            ss = small.tile([P_N, 1], f32, tag="ss")
            nc.scalar.activation(
                scr[:], kn_all[:, l * D : (l + 1) * D], AF.Square, accum_out=ss[:]
            )
            sq = small.tile([P_N, 1], f32, tag="sq")
            nc.scalar.activation(sq[:], ss[:], AF.Sqrt)
            y0 = small.tile([P_N, 1], f32, tag="y0")
            nc.vector.reciprocal(y0[:], sq[:])
            t1 = small.tile([P_N, 1], f32, tag="t1")
            nc.vector.tensor_tensor(t1[:], y0[:], y0[:], op=OP.mult)
            nc.vector.tensor_tensor(t1[:], t1[:], ss[:], op=OP.mult)
            nc.vector.tensor_scalar(t1[:], t1[:], -0.5, 1.5, OP.mult, OP.add)
            y1 = small.tile([P_N, 1], f32, tag="y1")
            nc.vector.tensor_tensor(y1[:], t1[:], y0[:], op=OP.mult)

            # Broadcast rinv across rows: rb = ones[100,128].T @ diag(rinv).
            # Each rb element is 99 exact zeros + 1.0*rinv_p, so bit-exact.
            dg = small.tile([P_N, P_N], f32, tag="dg")
            nc.vector.tensor_scalar_mul(dg[:], ident[:P_N, :P_N], y1[:])
            rb_ps = ps_rb.tile([128, P_N], f32, tag="rbp")
            nc.tensor.matmul(rb_ps[:], ones_col[:], dg[:], start=True, stop=True)
            rb = rowp.tile([128, P_N], f32, tag="rb")
            nc.scalar.copy(rb[:], rb_ps[:])

            # nkt = K^T * rinv (columns scaled): identical f32 products to
            # normalizing K rows and transposing.
            nkt = nktp.tile([128, C], f32, tag="nkt")
            base = l * C
            for j in range(NCH):
                nc.vector.tensor_tensor(
                    nkt[:, j * P_N : (j + 1) * P_N],
                    kt_all[:, base + j * P_N : base + (j + 1) * P_N],
                    rb[:],
                    op=OP.mult,
                )

            # ---- scores u = x_l @ nkt : psum [128b, 100p] ----
            pc = ps_pc.tile([BS, P_N], f32, tag="pc")
            for j in range(NCH):
                nc.tensor.matmul(
                    pc[:],
                    xt_all[:, (l * NCH + j) * 128 : (l * NCH + j + 1) * 128],
                    nkt[:, j * P_N : (j + 1) * P_N],
                    start=(j == 0),
                    stop=(j == NCH - 1),
                )
            u = rowp.tile([BS, P_N], f32, tag="u")
            nc.scalar.copy(u[:], pc[:])

            # ---- top-5 threshold via DVE max8 (comparison-only => exact) ----
            mm8 = small.tile([BS, 8], f32, tag="mm8")
            nc.vector.max(mm8[:], u[:])
            mask = rowp.tile([BS, P_N], f32, tag="mask")
            nc.vector.tensor_scalar(
                mask[:], u[:], mm8[:, TOP_K - 1 : TOP_K], None, OP.is_ge
            )

            # ---- s[p] = <K_hat, A_hat> via bf16 PE grams, diag by identity ----
            kb = kbp.tile([128, C], bf16, tag="kb")
            nc.gpsimd.tensor_copy(kb[:], nkt[:])
            gka = ps_ka.tile([P_N, P_N], f32, tag="gka")
            for j in range(NCH):
                nc.tensor.matmul(
                    gka[:],
                    kb[:, j * P_N : (j + 1) * P_N],
                    at_all[:, base + j * P_N : base + (j + 1) * P_N],
                    start=(j == 0),
                    stop=(j == NCH - 1),
                )
            gaa = ps_aa.tile([P_N, P_N], f32, tag="gaa")
            for j in range(NCH):
                nc.tensor.matmul(
                    gaa[:],
                    at_all[:, base + j * P_N : base + (j + 1) * P_N],
                    at_all[:, base + j * P_N : base + (j + 1) * P_N],
                    start=(j == 0),
                    stop=(j == NCH - 1),
                )
            # diag extraction: mask with identity, reduce rows.
            # (tensor_tensor_reduce reading PSUM is a HW crash - avoid.)
            dsc1 = scrp.tile([P_N, P_N], f32, tag="dscr")
            nc.vector.tensor_tensor(dsc1[:], gka[:], ident[:P_N, :P_N], op=OP.mult)
            ka_v = small.tile([P_N, 1], f32, tag="kav")
            nc.vector.reduce_sum(ka_v[:], dsc1[:], axis=AX.X)
            dsc2 = scrp.tile([P_N, P_N], f32, tag="dscr")
            nc.vector.tensor_tensor(dsc2[:], gaa[:], ident[:P_N, :P_N], op=OP.mult)
            aa_v = small.tile([P_N, 1], f32, tag="aav")
            nc.vector.reduce_sum(aa_v[:], dsc2[:], axis=AX.X)
            # kb is already K_hat, so s = ka_v / ||A||.
            sqa = small.tile([P_N, 1], f32, tag="sqa")
            nc.scalar.activation(sqa[:], aa_v[:], AF.Sqrt)
            ra = small.tile([P_N, 1], f32, tag="ra")
            nc.vector.reciprocal(ra[:], sqa[:])
            s_t = small.tile([P_N, 1], f32, tag="s_t")
            nc.vector.tensor_tensor(s_t[:], ka_v[:], ra[:], op=OP.mult)

            # W^T = mask^T * s -> [100, 128] bf16
            mt = ps_mt.tile([P_N, BS], f32, tag="mt")
            nc.tensor.transpose(mt[:], mask[:], ident[:])
            wt = wtp.tile([P_N, BS], bf16, tag="wt")
            nc.vector.tensor_scalar_mul(wt[:], mt[:], s_t[:])

            # ---- out[l] = W @ P : 12 x [128, 512] bf16 matmuls ----
            ob = obuf.tile([BS, NF], bf16, tag="ob")
            for n in range(12):
                po = ps_o.tile([BS, 512], f32, tag="po")
                nc.tensor.matmul(
                    po[:], wt[:], p_sb[:, n * 512 : (n + 1) * 512], start=True, stop=True
                )
                if n % 2 == 0:
                    nc.scalar.copy(ob[:, n * 512 : (n + 1) * 512], po[:])
                else:
                    nc.vector.tensor_copy(ob[:, n * 512 : (n + 1) * 512], po[:])
            nc.scalar.dma_start(o_d[l], ob[:])

    nc.compile()
    _CACHE["nc"] = nc
    return nc


def _pack_inputs(x_query, K_all, A_all, P_all):
    import ml_dtypes

    bf = ml_dtypes.bfloat16
    x = np.asarray(x_query, dtype=np.float32)
    K = np.asarray(K_all, dtype=np.float32)
    A = np.asarray(A_all, dtype=np.float32)
    P = np.asarray(P_all, dtype=np.float32)

    # x^T per core: [128dd, (l, j, b)]
    xt = np.ascontiguousarray(
        x.reshape(N_CORES, BS, L, NCH, 128).transpose(0, 4, 2, 3, 1).reshape(
            N_CORES, 128, L * D
        )
    )
    # K^T / A^T: [128dd, (l, j, p)]
    kt = np.ascontiguousarray(
        K.reshape(L, P_N, NCH, 128).transpose(3, 0, 2, 1).reshape(128, L * NCH * P_N)
    )
    at = np.ascontiguousarray(
        A.reshape(L, P_N, NCH, 128).transpose(3, 0, 2, 1).reshape(128, L * NCH * P_N)
    ).astype(bf)
    # K natural: [p, (l, d)]
    kn = np.ascontiguousarray(K.transpose(1, 0, 2).reshape(P_N, L * D))
    pp = np.ascontiguousarray(P.reshape(L, P_N, NF)).astype(bf)
    return xt, kt, at, kn, pp


def _run(x_query, K_all, A_all, P_all, trace=False, tmpdir=None):
    from concourse.bass_utils import run_bass_kernel_spmd

    xt, kt, at, kn, pp = _pack_inputs(x_query, K_all, A_all, P_all)
    nc = _build_nc()
    in_maps = [
        {"x": xt[c], "kt": kt, "at": at, "kn": kn, "p": pp} for c in range(N_CORES)
    ]
    br = run_bass_kernel_spmd(
        nc, in_maps, list(range(N_CORES)), trace=trace, tmpdir=tmpdir
    )
    out = np.stack([np.asarray(r["o"]) for r in br.results], axis=0)  # [8, L, BS, NF]
    out = out.astype(np.float32).transpose(1, 0, 2, 3).reshape(L, B, LP, D)
    return out, br


def kernel(x_query, K_all, A_all, P_all):
    out, _ = _run(x_query, K_all, A_all, P_all)
    return out


# revision 10
# speedup vs baseline: 1.6403x; 1.0511x over previous
"""Trainium2 Bass kernel for nn_L2PppMaskAttn (topk_masking).

Math reformulation of the reference:
  - top-5 ranking over prompts is invariant to q normalization, so scores
    u[b,p] = <x[b,l], K_hat[l,p]> suffice; mask = (u >= 5th_max(u)).
  - a_k depends only on (layer, prompt): s[l,p] = <K_hat[l,p], A_hat[l,p]>.
  - out[l,b] = (mask_row .* s) @ P_flat[l]: a [128,100] @ [100,6144] matmul.

Numerical contract: everything feeding the top-5 SELECTION replicates the
known-good op sequence bit-for-bit (scalar Square+accum for ||K||^2, sqrt,
reciprocal + one Newton step, f32 elementwise K*rinv products, PE f32
matmuls in the same 6x128-chunk accumulation order).  The tightest 5th/6th
score gap in this input set is ~1e-6 and a single flipped selection costs
~0.2 rel error, so this path must not be re-ordered.  The top-5 threshold
itself comes from DVE max8 (comparison-only, exact).  The output-scale path
(s, P matmul, store) runs in bf16: ~5e-3 worst-case vs the 2e-2 gate.

Layouts are host-packed so the device does no transposes and no casts:
  x^T   [128dd, (l,j,b)]          f32   4.7 MB   (j = 128-col chunk of D)
  K^T   [128dd, (l,j,p)]          f32   3.7 MB   (feeds scores)
  K     [100p,  (l,d)]            f32   3.7 MB   (feeds ||K||^2, exact path)
  KA^T  [128dd, (l,j,[K|A])]      bf16  3.7 MB   (feeds s via one PE gram)
  P     [l][100p, 6144]           bf16 14.8 MB
  out   [l][128b, 6144]           bf16 18.9 MB
~49.5 MB HBM traffic per core vs ~79 MB for the f32 baseline.  All loads
stream per-layer (double buffered) on the sync ring, P on the gpsimd ring,
stores on the scalar ring.

Sharding: data-parallel over batch, 8 cores x 128 rows; K/A/P replicated.
"""

import sys

sys.path.insert(0, "/opt/trn_rl_repo")

import numpy as np

B, L, P_N, LP, D = 1024, 12, 100, 8, 768
N_CORES = 8
BS = B // N_CORES  # 128 batch rows per core
NF = LP * D  # 6144 flattened output features per layer
NCH = D // 128  # 6 contraction chunks
C = NCH * P_N  # 600 K^T columns per layer
TOP_K = 5

_CACHE = {}


def _build_nc():
    if "nc" in _CACHE:
        return _CACHE["nc"]

    from contextlib import ExitStack

    import concourse.bass as bass
    import concourse.bacc as bacc
    import concourse.mybir as mybir
    from concourse import masks
    from concourse.tile import TileContext

    f32 = mybir.dt.float32
    bf16 = mybir.dt.bfloat16
    AX = mybir.AxisListType
    OP = mybir.AluOpType
    AF = mybir.ActivationFunctionType

    nc = bacc.Bacc(
        "TRN2",
        target_bir_lowering=False,
        debug=False,
        num_devices=N_CORES,
    )

    xt_d = nc.declare_dram_parameter("x", [128, L * D], f32, isOutput=False)
    kt_d = nc.declare_dram_parameter("kt", [128, L * C], f32, isOutput=False)
    ka_d = nc.declare_dram_parameter("ka", [128, L * 2 * C], bf16, isOutput=False)
    kn_d = nc.declare_dram_parameter("kn", [P_N, L * D], f32, isOutput=False)
    p_d = nc.declare_dram_parameter("p", [L, P_N, NF], bf16, isOutput=False)
    o_d = nc.declare_dram_parameter("o", [L, BS, NF], bf16, isOutput=True)

    with TileContext(nc) as tc, ExitStack() as ctx:
        pool = lambda name, bufs, **kw: ctx.enter_context(
            tc.tile_pool(name=name, bufs=bufs, **kw)
        )
        const = pool("const", 1)
        loadp = pool("loadp", 2)
        ppool = pool("pp", 3)
        nktp = pool("nktp", 2)
        scrp = pool("scrp", 2)
        rowp = pool("rowp", 2)
        small = pool("small", 2)
        wtp = pool("wtp", 2)
        obuf = pool("ob", 3)
        ps_pc = pool("ps_pc", 1, space="PSUM")
        ps_rb = pool("ps_rb", 1, space="PSUM")
        ps_g = pool("ps_g", 1, space="PSUM")
        ps_mt = pool("ps_mt", 1, space="PSUM")
        ps_o = pool("ps_o", 4, space="PSUM")

        ident = const.tile([128, 128], f32, tag="ident")
        masks.make_identity(nc, ident[:])
        ones_col = const.tile([100, 128], f32, tag="ones")
        nc.vector.memset(ones_col[:], 1.0)

        for l in range(L):
            # per-layer streams; kn first (rinv chain), then scores deps.
            kn_l = loadp.tile([P_N, D], f32, tag="kn")
            nc.sync.dma_start(kn_l[:], kn_d[:, l * D : (l + 1) * D])
            kt_l = loadp.tile([128, C], f32, tag="kt")
            nc.sync.dma_start(kt_l[:], kt_d[:, l * C : (l + 1) * C])
            xt_l = loadp.tile([128, D], f32, tag="xt")
            nc.sync.dma_start(xt_l[:], xt_d[:, l * D : (l + 1) * D])
            ka_l = loadp.tile([128, 2 * C], bf16, tag="ka")
            nc.sync.dma_start(ka_l[:], ka_d[:, l * 2 * C : (l + 1) * 2 * C])
            p_sb = ppool.tile([P_N, NF], bf16, tag="p")
            nc.gpsimd.dma_start(p_sb[:], p_d[l])

            # ---- rinv = 1/||K_p|| (selection-critical, baseline op order) ----
            scr = scrp.tile([P_N, D], f32, tag="scr")
            ss = small.tile([P_N, 1], f32, tag="ss")
            nc.scalar.activation(scr[:], kn_l[:], AF.Square, accum_out=ss[:])
            sq = small.tile([P_N, 1], f32, tag="sq")
            nc.scalar.activation(sq[:], ss[:], AF.Sqrt)
            y0 = small.tile([P_N, 1], f32, tag="y0")
            nc.vector.reciprocal(y0[:], sq[:])
            t1 = small.tile([P_N, 1], f32, tag="t1")
            nc.vector.tensor_tensor(t1[:], y0[:], y0[:], op=OP.mult)
            nc.vector.tensor_tensor(t1[:], t1[:], ss[:], op=OP.mult)
            nc.vector.tensor_scalar(t1[:], t1[:], -0.5, 1.5, OP.mult, OP.add)
            y1 = small.tile([P_N, 1], f32, tag="y1")
            nc.vector.tensor_tensor(y1[:], t1[:], y0[:], op=OP.mult)

            # Broadcast rinv across rows: rb = ones[100,128].T @ diag(rinv).
            # Each rb element is 99 exact zeros + 1.0*rinv_p, so bit-exact.
            dg = small.tile([P_N, P_N], f32, tag="dg")
            nc.vector.tensor_scalar_mul(dg[:], ident[:P_N, :P_N], y1[:])
            rb_ps = ps_rb.tile([128, P_N], f32, tag="rbp")
            nc.tensor.matmul(rb_ps[:], ones_col[:], dg[:], start=True, stop=True)
            rb = rowp.tile([128, P_N], f32, tag="rb")
            nc.scalar.copy(rb[:], rb_ps[:])

            # nkt = K^T * rinv (columns scaled): identical f32 products to
            # normalizing K rows and transposing.
            nkt = nktp.tile([128, C], f32, tag="nkt")
            for j in range(NCH):
                nc.vector.tensor_tensor(
                    nkt[:, j * P_N : (j + 1) * P_N],
                    kt_l[:, j * P_N : (j + 1) * P_N],
                    rb[:],
                    op=OP.mult,
                )

            # ---- scores u = x_l @ nkt : psum [128b, 100p] ----
            pc = ps_pc.tile([BS, P_N], f32, tag="pc")
            for j in range(NCH):
                nc.tensor.matmul(
                    pc[:],
                    xt_l[:, j * 128 : (j + 1) * 128],
                    nkt[:, j * P_N : (j + 1) * P_N],
                    start=(j == 0),
                    stop=(j == NCH - 1),
                )
            u = rowp.tile([BS, P_N], f32, tag="u")
            nc.scalar.copy(u[:], pc[:])

            # ---- top-5 threshold via DVE max8 (comparison-only => exact) ----
            mm8 = small.tile([BS, 8], f32, tag="mm8")
            nc.vector.max(mm8[:], u[:])
            mask = rowp.tile([BS, P_N], f32, tag="mask")
            nc.vector.tensor_scalar(
                mask[:], u[:], mm8[:, TOP_K - 1 : TOP_K], None, OP.is_ge
            )

            # ---- s[p] = <K,A>/(||K||*||A||) via one bf16 PE gram ----
            # gg = A^T_j' @ [K^T_j | A^T_j] accumulated over j:
            #   cols 0:100 = A@K^T (diag -> <K_p,A_p>), 100:200 = A@A^T.
            gg = ps_g.tile([P_N, 2 * P_N], f32, tag="gg")
            for j in range(NCH):
                nc.tensor.matmul(
                    gg[:],
                    ka_l[:, j * 2 * P_N + P_N : (j + 1) * 2 * P_N],
                    ka_l[:, j * 2 * P_N : (j + 1) * 2 * P_N],
                    start=(j == 0),
                    stop=(j == NCH - 1),
                )
            dsc1 = scrp.tile([P_N, P_N], f32, tag="dscr")
            nc.vector.tensor_tensor(dsc1[:], gg[:, :P_N], ident[:P_N, :P_N], op=OP.mult)
            ka_v = small.tile([P_N, 1], f32, tag="kav")
            nc.vector.reduce_sum(ka_v[:], dsc1[:], axis=AX.X)
            dsc2 = scrp.tile([P_N, P_N], f32, tag="dscr")
            nc.vector.tensor_tensor(dsc2[:], gg[:, P_N:], ident[:P_N, :P_N], op=OP.mult)
            aa_v = small.tile([P_N, 1], f32, tag="aav")
            nc.vector.reduce_sum(aa_v[:], dsc2[:], axis=AX.X)
            # s = <K,A> * rinv_K / ||A||
            sqa = small.tile([P_N, 1], f32, tag="sqa")
            nc.scalar.activation(sqa[:], aa_v[:], AF.Sqrt)
            ra = small.tile([P_N, 1], f32, tag="ra")
            nc.vector.reciprocal(ra[:], sqa[:])
            s_t = small.tile([P_N, 1], f32, tag="s_t")
            nc.vector.tensor_tensor(s_t[:], ka_v[:], ra[:], op=OP.mult)
            nc.vector.tensor_tensor(s_t[:], s_t[:], y1[:], op=OP.mult)

            # W^T = mask^T * s -> [100, 128] bf16
            mt = ps_mt.tile([P_N, BS], f32, tag="mt")
            nc.tensor.transpose(mt[:], mask[:], ident[:])
            wt = wtp.tile([P_N, BS], bf16, tag="wt")
            nc.vector.tensor_scalar_mul(wt[:], mt[:], s_t[:])

            # ---- out[l] = W @ P : 12 x [128, 512] bf16 matmuls ----
            ob = obuf.tile([BS, NF], bf16, tag="ob")
            for n in range(12):
                po = ps_o.tile([BS, 512], f32, tag="po")
                nc.tensor.matmul(
                    po[:], wt[:], p_sb[:, n * 512 : (n + 1) * 512], start=True, stop=True
                )
                if n % 2 == 0:
                    nc.scalar.copy(ob[:, n * 512 : (n + 1) * 512], po[:])
                else:
                    nc.vector.tensor_copy(ob[:, n * 512 : (n + 1) * 512], po[:])
            nc.scalar.dma_start(o_d[l], ob[:])

    nc.compile()
    _CACHE["nc"] = nc
    return nc


def _pack_inputs(x_query, K_all, A_all, P_all):
    import ml_dtypes

    bf = ml_dtypes.bfloat16
    x = np.asarray(x_query, dtype=np.float32)
    K = np.asarray(K_all, dtype=np.float32)
    A = np.asarray(A_all, dtype=np.float32)
    P = np.asarray(P_all, dtype=np.float32)

    # x^T per core: [128dd, (l, j, b)]
    xt = np.ascontiguousarray(
        x.reshape(N_CORES, BS, L, NCH, 128).transpose(0, 4, 2, 3, 1).reshape(
            N_CORES, 128, L * D
        )
    )
    # K^T: [128dd, (l, j, p)] f32
    kt6 = K.reshape(L, P_N, NCH, 128).transpose(3, 0, 2, 1)  # [128, L, 6, 100]
    kt = np.ascontiguousarray(kt6.reshape(128, L * C))
    # interleaved [K^T | A^T] bf16: [128dd, (l, j, [K100 | A100])]
    at6 = A.reshape(L, P_N, NCH, 128).transpose(3, 0, 2, 1)
    ka = np.empty((128, L, NCH, 2, P_N), dtype=np.float32)
    ka[:, :, :, 0, :] = kt6
    ka[:, :, :, 1, :] = at6
    ka = np.ascontiguousarray(ka.reshape(128, L * 2 * C)).astype(bf)
    # K natural: [p, (l, d)]
    kn = np.ascontiguousarray(K.transpose(1, 0, 2).reshape(P_N, L * D))
    pp = np.ascontiguousarray(P.reshape(L, P_N, NF)).astype(bf)
    return xt, kt, ka, kn, pp


def _run(x_query, K_all, A_all, P_all, trace=False, tmpdir=None):
    from concourse.bass_utils import run_bass_kernel_spmd

    xt, kt, ka, kn, pp = _pack_inputs(x_query, K_all, A_all, P_all)
    nc = _build_nc()
    in_maps = [
        {"x": xt[c], "kt": kt, "ka": ka, "kn": kn, "p": pp} for c in range(N_CORES)
    ]
    br = run_bass_kernel_spmd(
        nc, in_maps, list(range(N_CORES)), trace=trace, tmpdir=tmpdir
    )
    out = np.stack([np.asarray(r["o"]) for r in br.results], axis=0)  # [8, L, BS, NF]
    out = out.astype(np.float32).transpose(1, 0, 2, 3).reshape(L, B, LP, D)
    return out, br


def kernel(x_query, K_all, A_all, P_all):
    out, _ = _run(x_query, K_all, A_all, P_all)
    return out


# revision 17
# speedup vs baseline: 1.6646x; 1.0148x over previous
"""Trainium2 Bass kernel for nn_L2PppMaskAttn (topk_masking).

Math reformulation of the reference:
  - top-5 ranking over prompts is invariant to q normalization, so scores
    u[b,p] = <x[b,l], K_hat[l,p]> suffice; mask = (u >= 5th_max(u)).
  - a_k depends only on (layer, prompt): s[l,p] = <K_hat[l,p], A_hat[l,p]>.
  - out[l,b] = (mask_row .* s) @ P_flat[l]: a [128,100] @ [100,6144] matmul.

Numerical contract: everything feeding the top-5 SELECTION replicates the
known-good op sequence bit-for-bit (scalar Square+accum for ||K||^2, sqrt,
reciprocal + one Newton step, f32 elementwise K*rinv products, PE f32
matmuls in the same 6x128-chunk accumulation order).  The tightest 5th/6th
score gap in this input set is ~1e-6 and a single flipped selection costs
~0.2 rel error, so this path must not be re-ordered.  The top-5 threshold
itself comes from DVE max8 (comparison-only, exact).  The output-scale path
(s, P matmul, store) runs in bf16: ~5e-3 worst-case vs the 2e-2 gate.

Layouts are host-packed so the device does no transposes and no casts:
  x^T   [128dd, (l,j,b)]          f32   4.7 MB   (j = 128-col chunk of D)
  K^T   [128dd, (l,j,p)]          f32   3.7 MB   (feeds scores)
  K     [100p,  (l,d)]            f32   3.7 MB   (feeds ||K||^2, exact path)
  KA^T  [128dd, (l,j,[K|A])]      bf16  3.7 MB   (feeds s via one PE gram)
  P     [l][100p, 6144]           bf16 14.8 MB
  out   [l][128b, 6144]           bf16 18.9 MB
~49.5 MB HBM traffic per core vs ~79 MB for the f32 baseline.  All loads
stream per-layer (double buffered) on the sync ring, P on the gpsimd ring,
stores on the scalar ring.

Sharding: data-parallel over batch, 8 cores x 128 rows; K/A/P replicated.
"""

import sys

sys.path.insert(0, "/opt/trn_rl_repo")

import numpy as np

B, L, P_N, LP, D = 1024, 12, 100, 8, 768
N_CORES = 8
BS = B // N_CORES  # 128 batch rows per core
NF = LP * D  # 6144 flattened output features per layer
NCH = D // 128  # 6 contraction chunks
C = NCH * P_N  # 600 K^T columns per layer
TOP_K = 5

_CACHE = {}


def _build_nc():
    if "nc" in _CACHE:
        return _CACHE["nc"]

    from contextlib import ExitStack

    import concourse.bass as bass
    import concourse.bacc as bacc
    import concourse.mybir as mybir
    from concourse import masks
    from concourse.tile import TileContext

    f32 = mybir.dt.float32
    bf16 = mybir.dt.bfloat16
    AX = mybir.AxisListType
    OP = mybir.AluOpType
    AF = mybir.ActivationFunctionType

    nc = bacc.Bacc(
        "TRN2",
        target_bir_lowering=False,
        debug=False,
        num_devices=N_CORES,
    )

    xt_d = nc.declare_dram_parameter("x", [128, L * D], f32, isOutput=False)
    kt_d = nc.declare_dram_parameter("kt", [128, L * C], f32, isOutput=False)
    ka_d = nc.declare_dram_parameter("ka", [128, L * 2 * C], bf16, isOutput=False)
    kn_d = nc.declare_dram_parameter("kn", [P_N, L * D], f32, isOutput=False)
    p_d = nc.declare_dram_parameter("p", [L, P_N, NF], bf16, isOutput=False)
    o_d = nc.declare_dram_parameter("o", [L, BS, NF], bf16, isOutput=True)

    with TileContext(nc) as tc, ExitStack() as ctx:
        pool = lambda name, bufs, **kw: ctx.enter_context(
            tc.tile_pool(name=name, bufs=bufs, **kw)
        )
        const = pool("const", 1)
        loadp = pool("loadp", 2)
        ppool = pool("pp", 2)
        nktp = pool("nktp", 2)
        scrp = pool("scrp", 2)
        rowp = pool("rowp", 2)
        small = pool("small", 2)
        wtp = pool("wtp", 2)
        obuf = pool("ob", 3)
        ps_pc = pool("ps_pc", 1, space="PSUM")
        ps_rb = pool("ps_rb", 1, space="PSUM")
        ps_g = pool("ps_g", 1, space="PSUM")
        ps_mt = pool("ps_mt", 1, space="PSUM")
        ps_o = pool("ps_o", 4, space="PSUM")

        ident = const.tile([128, 128], f32, tag="ident")
        masks.make_identity(nc, ident[:])
        ones_col = const.tile([100, 128], f32, tag="ones")
        nc.vector.memset(ones_col[:], 1.0)

        # kn resident: layer-0 slice lands in ~2us so the rinv chain starts
        # immediately; the rest follows chunk 0 as one efficient transfer.
        kn_all = const.tile([P_N, L * D], f32, tag="kn")
        nc.sync.dma_start(kn_all[:, :D], kn_d[:, :D])

        def _layer(l, li, kt_c, xt_c, ka_c, p_sb):
            # ---- rinv = 1/||K_p|| (selection-critical, baseline op order) ----
            scr = scrp.tile([P_N, D], f32, tag="scr")
            ss = small.tile([P_N, 1], f32, tag="ss")
            nc.scalar.activation(
                scr[:], kn_all[:, l * D : (l + 1) * D], AF.Square, accum_out=ss[:]
            )
            sq = small.tile([P_N, 1], f32, tag="sq")
            nc.scalar.activation(sq[:], ss[:], AF.Sqrt)
            y0 = small.tile([P_N, 1], f32, tag="y0")
            nc.vector.reciprocal(y0[:], sq[:])
            t1 = small.tile([P_N, 1], f32, tag="t1")
            nc.vector.tensor_tensor(t1[:], y0[:], y0[:], op=OP.mult)
            nc.vector.tensor_tensor(t1[:], t1[:], ss[:], op=OP.mult)
            nc.vector.tensor_scalar(t1[:], t1[:], -0.5, 1.5, OP.mult, OP.add)
            y1 = small.tile([P_N, 1], f32, tag="y1")
            nc.vector.tensor_tensor(y1[:], t1[:], y0[:], op=OP.mult)

            # Broadcast rinv across rows: rb = ones[100,128].T @ diag(rinv).
            # Each rb element is 99 exact zeros + 1.0*rinv_p, so bit-exact.
            dg = small.tile([P_N, P_N], f32, tag="dg")
            nc.vector.tensor_scalar_mul(dg[:], ident[:P_N, :P_N], y1[:])
            rb_ps = ps_rb.tile([128, P_N], f32, tag="rbp")
            nc.tensor.matmul(rb_ps[:], ones_col[:], dg[:], start=True, stop=True)
            rb = rowp.tile([128, P_N], f32, tag="rb")
            nc.scalar.copy(rb[:], rb_ps[:])

            # nkt = K^T * rinv (columns scaled): identical f32 products to
            # normalizing K rows and transposing.
            kt0 = li * C
            nkt = nktp.tile([128, C], f32, tag="nkt")
            for j in range(NCH):
                nc.vector.tensor_tensor(
                    nkt[:, j * P_N : (j + 1) * P_N],
                    kt_c[:, kt0 + j * P_N : kt0 + (j + 1) * P_N],
                    rb[:],
                    op=OP.mult,
                )

            # ---- scores u = x_l @ nkt : psum [128b, 100p] ----
            xt0 = li * D
            pc = ps_pc.tile([BS, P_N], f32, tag="pc")
            for j in range(NCH):
                nc.tensor.matmul(
                    pc[:],
                    xt_c[:, xt0 + j * 128 : xt0 + (j + 1) * 128],
                    nkt[:, j * P_N : (j + 1) * P_N],
                    start=(j == 0),
                    stop=(j == NCH - 1),
                )
            u = rowp.tile([BS, P_N], f32, tag="u")
            nc.scalar.copy(u[:], pc[:])

            # ---- top-5 threshold via DVE max8 (comparison-only => exact) ----
            mm8 = small.tile([BS, 8], f32, tag="mm8")
            nc.vector.max(mm8[:], u[:])
            mask = rowp.tile([BS, P_N], f32, tag="mask")
            nc.vector.tensor_scalar(
                mask[:], u[:], mm8[:, TOP_K - 1 : TOP_K], None, OP.is_ge
            )

            # ---- s[p] = <K,A>/(||K||*||A||) via one bf16 PE gram ----
            # gg = A^T_j' @ [K^T_j | A^T_j] accumulated over j:
            #   cols 0:100 = A@K^T (diag -> <K_p,A_p>), 100:200 = A@A^T.
            ka0 = li * 2 * C
            gg = ps_g.tile([P_N, 2 * P_N], f32, tag="gg")
            for j in range(NCH):
                nc.tensor.matmul(
                    gg[:],
                    ka_c[:, ka0 + j * 2 * P_N + P_N : ka0 + (j + 1) * 2 * P_N],
                    ka_c[:, ka0 + j * 2 * P_N : ka0 + (j + 1) * 2 * P_N],
                    start=(j == 0),
                    stop=(j == NCH - 1),
                )
            dsc1 = scrp.tile([P_N, P_N], f32, tag="dscr")
            nc.vector.tensor_tensor(dsc1[:], gg[:, :P_N], ident[:P_N, :P_N], op=OP.mult)
            ka_v = small.tile([P_N, 1], f32, tag="kav")
            nc.vector.reduce_sum(ka_v[:], dsc1[:], axis=AX.X)
            dsc2 = scrp.tile([P_N, P_N], f32, tag="dscr")
            nc.vector.tensor_tensor(dsc2[:], gg[:, P_N:], ident[:P_N, :P_N], op=OP.mult)
            aa_v = small.tile([P_N, 1], f32, tag="aav")
            nc.vector.reduce_sum(aa_v[:], dsc2[:], axis=AX.X)
            # s = <K,A> * rinv_K / ||A||
            sqa = small.tile([P_N, 1], f32, tag="sqa")
            nc.scalar.activation(sqa[:], aa_v[:], AF.Sqrt)
            ra = small.tile([P_N, 1], f32, tag="ra")
            nc.vector.reciprocal(ra[:], sqa[:])
            s_t = small.tile([P_N, 1], f32, tag="s_t")
            nc.vector.tensor_tensor(s_t[:], ka_v[:], ra[:], op=OP.mult)
            nc.vector.tensor_tensor(s_t[:], s_t[:], y1[:], op=OP.mult)

            # W^T = mask^T * s -> [100, 128] bf16
            mt = ps_mt.tile([P_N, BS], f32, tag="mt")
            nc.tensor.transpose(mt[:], mask[:], ident[:])
            wt = wtp.tile([P_N, BS], bf16, tag="wt")
            nc.vector.tensor_scalar_mul(wt[:], mt[:], s_t[:])

            # ---- out[l] = W @ P : 12 x [128, 512] bf16 matmuls ----
            ob = obuf.tile([BS, NF], bf16, tag="ob")
            for n in range(12):
                po = ps_o.tile([BS, 512], f32, tag="po")
                nc.tensor.matmul(
                    po[:], wt[:], p_sb[:, n * 512 : (n + 1) * 512], start=True, stop=True
                )
                if n % 2 == 0:
                    nc.scalar.copy(ob[:, n * 512 : (n + 1) * 512], po[:])
                else:
                    nc.vector.tensor_copy(ob[:, n * 512 : (n + 1) * 512], po[:])
            nc.scalar.dma_start(o_d[l], ob[:])

        LPC = 3  # layers per load chunk: ~1 MB per DMA for decent efficiency
        for lc in range(L // LPC):
            kt_c = loadp.tile([128, LPC * C], f32, tag="kt")
            nc.sync.dma_start(kt_c[:], kt_d[:, lc * LPC * C : (lc + 1) * LPC * C])
            xt_c = loadp.tile([128, LPC * D], f32, tag="xt")
            nc.sync.dma_start(xt_c[:], xt_d[:, lc * LPC * D : (lc + 1) * LPC * D])
            ka_c = loadp.tile([128, LPC * 2 * C], bf16, tag="ka")
            nc.sync.dma_start(
                ka_c[:], ka_d[:, lc * LPC * 2 * C : (lc + 1) * LPC * 2 * C]
            )
            if lc == 0:
                nc.sync.dma_start(kn_all[:, D:], kn_d[:, D:])
            for li in range(LPC):
                l = lc * LPC + li
                p_sb = ppool.tile([P_N, NF], bf16, tag="p")
                nc.gpsimd.dma_start(p_sb[:], p_d[l])
                _layer(l, li, kt_c, xt_c, ka_c, p_sb)

    nc.compile()
    _CACHE["nc"] = nc
    return nc


def _pack_inputs(x_query, K_all, A_all, P_all):
    import ml_dtypes

    bf = ml_dtypes.bfloat16
    x = np.asarray(x_query, dtype=np.float32)
    K = np.asarray(K_all, dtype=np.float32)
    A = np.asarray(A_all, dtype=np.float32)
    P = np.asarray(P_all, dtype=np.float32)

    # x^T per core: [128dd, (l, j, b)]
    xt = np.ascontiguousarray(
        x.reshape(N_CORES, BS, L, NCH, 128).transpose(0, 4, 2, 3, 1).reshape(
            N_CORES, 128, L * D
        )
    )
    # K^T: [128dd, (l, j, p)] f32
    kt6 = K.reshape(L, P_N, NCH, 128).transpose(3, 0, 2, 1)  # [128, L, 6, 100]
    kt = np.ascontiguousarray(kt6.reshape(128, L * C))
    # interleaved [K^T | A^T] bf16: [128dd, (l, j, [K100 | A100])]
    at6 = A.reshape(L, P_N, NCH, 128).transpose(3, 0, 2, 1)
    ka = np.empty((128, L, NCH, 2, P_N), dtype=np.float32)
    ka[:, :, :, 0, :] = kt6
    ka[:, :, :, 1, :] = at6
    ka = np.ascontiguousarray(ka.reshape(128, L * 2 * C)).astype(bf)
    # K natural: [p, (l, d)]
    kn = np.ascontiguousarray(K.transpose(1, 0, 2).reshape(P_N, L * D))
    pp = np.ascontiguousarray(P.reshape(L, P_N, NF)).astype(bf)
    return xt, kt, ka, kn, pp


def _run(x_query, K_all, A_all, P_all, trace=False, tmpdir=None):
    from concourse.bass_utils import run_bass_kernel_spmd

    xt, kt, ka, kn, pp = _pack_inputs(x_query, K_all, A_all, P_all)
    nc = _build_nc()
    in_maps = [
        {"x": xt[c], "kt": kt, "ka": ka, "kn": kn, "p": pp} for c in range(N_CORES)
    ]
    br = run_bass_kernel_spmd(
        nc, in_maps, list(range(N_CORES)), trace=trace, tmpdir=tmpdir
    )
    out = np.stack([np.asarray(r["o"]) for r in br.results], axis=0)  # [8, L, BS, NF]
    out = out.astype(np.float32).transpose(1, 0, 2, 3).reshape(L, B, LP, D)
    return out, br


def kernel(x_query, K_all, A_all, P_all):
    out, _ = _run(x_query, K_all, A_all, P_all)
    return out


# revision 18
# speedup vs baseline: 1.6767x; 1.0073x over previous
"""Trainium2 Bass kernel for nn_L2PppMaskAttn (topk_masking).

Math reformulation of the reference:
  - top-5 ranking over prompts is invariant to q normalization, so scores
    u[b,p] = <x[b,l], K_hat[l,p]> suffice; mask = (u >= 5th_max(u)).
  - a_k depends only on (layer, prompt): s[l,p] = <K_hat[l,p], A_hat[l,p]>.
  - out[l,b] = (mask_row .* s) @ P_flat[l]: a [128,100] @ [100,6144] matmul.

Numerical contract: everything feeding the top-5 SELECTION replicates the
known-good op sequence bit-for-bit (scalar Square+accum for ||K||^2, sqrt,
reciprocal + one Newton step, f32 elementwise K*rinv products, PE f32
matmuls in the same 6x128-chunk accumulation order).  The tightest 5th/6th
score gap in this input set is ~1e-6 and a single flipped selection costs
~0.2 rel error, so this path must not be re-ordered.  The top-5 threshold
itself comes from DVE max8 (comparison-only, exact).  The output-scale path
(s, P matmul, store) runs in bf16: ~5e-3 worst-case vs the 2e-2 gate.

Two-phase schedule: phase 1 computes selection weights W^T[l] for ALL
layers (cheap serial chains, overlapped with the big input loads); phase 2
is a pure streaming loop - per layer 12 bf16 matmuls, PSUM->SBUF casts and
one 1.5 MB store - that keeps the PE warm and the store/P DMA rings full.

Host-packed layouts (no device transposes):
  x^T  [128dd, (l,j,b)]   f32   4.7 MB      K^T  [128dd, (l,j,p)]  f32  3.7 MB
  K    [100p, (l,d)]      f32   3.7 MB      A^T  [128dd, (l,j,p)]  bf16 1.8 MB
  P    [l][100p, 6144]    bf16 14.8 MB      out  [l][128b, 6144]   bf16 18.9 MB
~47.7 MB HBM traffic per core vs ~79 MB for the f32 baseline.

Sharding: data-parallel over batch, 8 cores x 128 rows; K/A/P replicated.
"""

import sys

sys.path.insert(0, "/opt/trn_rl_repo")

import numpy as np

B, L, P_N, LP, D = 1024, 12, 100, 8, 768
N_CORES = 8
BS = B // N_CORES  # 128 batch rows per core
NF = LP * D  # 6144 flattened output features per layer
NCH = D // 128  # 6 contraction chunks
C = NCH * P_N  # 600 K^T columns per layer
TOP_K = 5
LPC = 4  # layers per load chunk (~1.2 MB per DMA)

_CACHE = {}


def _build_nc():
    if "nc" in _CACHE:
        return _CACHE["nc"]

    from contextlib import ExitStack

    import concourse.bass as bass
    import concourse.bacc as bacc
    import concourse.mybir as mybir
    from concourse import masks
    from concourse.tile import TileContext

    f32 = mybir.dt.float32
    bf16 = mybir.dt.bfloat16
    AX = mybir.AxisListType
    OP = mybir.AluOpType
    AF = mybir.ActivationFunctionType

    nc = bacc.Bacc(
        "TRN2",
        target_bir_lowering=False,
        debug=False,
        num_devices=N_CORES,
    )

    xt_d = nc.declare_dram_parameter("x", [128, L * D], f32, isOutput=False)
    kt_d = nc.declare_dram_parameter("kt", [128, L * C], f32, isOutput=False)
    at_d = nc.declare_dram_parameter("at", [128, L * C], bf16, isOutput=False)
    kn_d = nc.declare_dram_parameter("kn", [P_N, L * D], f32, isOutput=False)
    p_d = nc.declare_dram_parameter("p", [L, P_N, NF], bf16, isOutput=False)
    o_d = nc.declare_dram_parameter("o", [L, BS, NF], bf16, isOutput=True)

    with TileContext(nc) as tc, ExitStack() as ctx:
        pool = lambda name, bufs, **kw: ctx.enter_context(
            tc.tile_pool(name=name, bufs=bufs, **kw)
        )
        const = pool("const", 1)
        loadp = pool("loadp", 2)
        ppool = pool("pp", 3)
        nktp = pool("nktp", 2)
        kbp = pool("kbp", 2)
        scrp = pool("scrp", 2)
        rowp = pool("rowp", 2)
        small = pool("small", 2)
        obuf = pool("ob", 3)
        ps_pc = pool("ps_pc", 1, space="PSUM")
        ps_rb = pool("ps_rb", 1, space="PSUM")
        ps_ka = pool("ps_ka", 1, space="PSUM")
        ps_aa = pool("ps_aa", 1, space="PSUM")
        ps_mt = pool("ps_mt", 1, space="PSUM")
        ps_o = pool("ps_o", 3, space="PSUM")

        ident = const.tile([128, 128], f32, tag="ident")
        masks.make_identity(nc, ident[:])
        ones_col = const.tile([100, 128], f32, tag="ones")
        nc.vector.memset(ones_col[:], 1.0)
        wt_all = const.tile([P_N, L * BS], bf16, tag="wt")

        # kn resident: layer-0 slice lands first so the rinv chain starts
        # immediately; the rest follows chunk 0 as one efficient transfer.
        kn_all = const.tile([P_N, L * D], f32, tag="kn")
        nc.sync.dma_start(kn_all[:, :D], kn_d[:, :D])

        def _sel(l, li, kt_c, xt_c, at_c):
            """Selection + weights for layer l -> wt_all[:, l*BS:(l+1)*BS]."""
            # rinv = 1/||K_p|| (selection-critical, baseline op order)
            scr = scrp.tile([P_N, D], f32, tag="scr")
            ss = small.tile([P_N, 1], f32, tag="ss")
            nc.scalar.activation(
                scr[:], kn_all[:, l * D : (l + 1) * D], AF.Square, accum_out=ss[:]
            )
            sq = small.tile([P_N, 1], f32, tag="sq")
            nc.scalar.activation(sq[:], ss[:], AF.Sqrt)
            y0 = small.tile([P_N, 1], f32, tag="y0")
            nc.vector.reciprocal(y0[:], sq[:])
            t1 = small.tile([P_N, 1], f32, tag="t1")
            nc.vector.tensor_tensor(t1[:], y0[:], y0[:], op=OP.mult)
            nc.vector.tensor_tensor(t1[:], t1[:], ss[:], op=OP.mult)
            nc.vector.tensor_scalar(t1[:], t1[:], -0.5, 1.5, OP.mult, OP.add)
            y1 = small.tile([P_N, 1], f32, tag="y1")
            nc.vector.tensor_tensor(y1[:], t1[:], y0[:], op=OP.mult)

            # Broadcast rinv across rows: rb = ones[100,128].T @ diag(rinv).
            # Each rb element is 99 exact zeros + 1.0*rinv_p, so bit-exact.
            dg = small.tile([P_N, P_N], f32, tag="dg")
            nc.vector.tensor_scalar_mul(dg[:], ident[:P_N, :P_N], y1[:])
            rb_ps = ps_rb.tile([128, P_N], f32, tag="rbp")
            nc.tensor.matmul(rb_ps[:], ones_col[:], dg[:], start=True, stop=True)
            rb = rowp.tile([128, P_N], f32, tag="rb")
            nc.scalar.copy(rb[:], rb_ps[:])

            # nkt = K^T * rinv (columns scaled): identical f32 products to
            # normalizing K rows and transposing.
            kt0 = li * C
            nkt = nktp.tile([128, C], f32, tag="nkt")
            for j in range(NCH):
                nc.vector.tensor_tensor(
                    nkt[:, j * P_N : (j + 1) * P_N],
                    kt_c[:, kt0 + j * P_N : kt0 + (j + 1) * P_N],
                    rb[:],
                    op=OP.mult,
                )

            # scores u = x_l @ nkt : psum [128b, 100p]
            xt0 = li * D
            pc = ps_pc.tile([BS, P_N], f32, tag="pc")
            for j in range(NCH):
                nc.tensor.matmul(
                    pc[:],
                    xt_c[:, xt0 + j * 128 : xt0 + (j + 1) * 128],
                    nkt[:, j * P_N : (j + 1) * P_N],
                    start=(j == 0),
                    stop=(j == NCH - 1),
                )
            u = rowp.tile([BS, P_N], f32, tag="u")
            nc.scalar.copy(u[:], pc[:])

            # top-5 threshold via DVE max8 (comparison-only => exact)
            mm8 = small.tile([BS, 8], f32, tag="mm8")
            nc.vector.max(mm8[:], u[:])
            mask = rowp.tile([BS, P_N], f32, tag="mask")
            nc.vector.tensor_scalar(
                mask[:], u[:], mm8[:, TOP_K - 1 : TOP_K], None, OP.is_ge
            )

            # s[p] = <K_hat, A_hat>: bf16 grams with K_hat cast on gpsimd
            kb = kbp.tile([128, C], bf16, tag="kb")
            nc.gpsimd.tensor_copy(kb[:], nkt[:])
            at0 = li * C
            gka = ps_ka.tile([P_N, P_N], f32, tag="gka")
            for j in range(NCH):
                nc.tensor.matmul(
                    gka[:],
                    at_c[:, at0 + j * P_N : at0 + (j + 1) * P_N],
                    kb[:, j * P_N : (j + 1) * P_N],
                    start=(j == 0),
                    stop=(j == NCH - 1),
                )
            gaa = ps_aa.tile([P_N, P_N], f32, tag="gaa")
            for j in range(NCH):
                nc.tensor.matmul(
                    gaa[:],
                    at_c[:, at0 + j * P_N : at0 + (j + 1) * P_N],
                    at_c[:, at0 + j * P_N : at0 + (j + 1) * P_N],
                    start=(j == 0),
                    stop=(j == NCH - 1),
                )
            dsc1 = scrp.tile([P_N, P_N], f32, tag="dscr")
            nc.vector.tensor_tensor(dsc1[:], gka[:], ident[:P_N, :P_N], op=OP.mult)
            ka_v = small.tile([P_N, 1], f32, tag="kav")
            nc.vector.reduce_sum(ka_v[:], dsc1[:], axis=AX.X)
            dsc2 = scrp.tile([P_N, P_N], f32, tag="dscr")
            nc.vector.tensor_tensor(dsc2[:], gaa[:], ident[:P_N, :P_N], op=OP.mult)
            aa_v = small.tile([P_N, 1], f32, tag="aav")
            nc.vector.reduce_sum(aa_v[:], dsc2[:], axis=AX.X)
            sqa = small.tile([P_N, 1], f32, tag="sqa")
            nc.scalar.activation(sqa[:], aa_v[:], AF.Sqrt)
            ra = small.tile([P_N, 1], f32, tag="ra")
            nc.vector.reciprocal(ra[:], sqa[:])
            s_t = small.tile([P_N, 1], f32, tag="s_t")
            nc.vector.tensor_tensor(s_t[:], ka_v[:], ra[:], op=OP.mult)

            # W^T = mask^T * s -> wt_all columns
            mt = ps_mt.tile([P_N, BS], f32, tag="mt")
            nc.tensor.transpose(mt[:], mask[:], ident[:])
            nc.vector.tensor_scalar_mul(
                wt_all[:, l * BS : (l + 1) * BS], mt[:], s_t[:]
            )

        # ---- phase 1: chunked loads + all selection work ----
        for lc in range(L // LPC):
            kt_c = loadp.tile([128, LPC * C], f32, tag="kt")
            nc.sync.dma_start(kt_c[:], kt_d[:, lc * LPC * C : (lc + 1) * LPC * C])
            xt_c = loadp.tile([128, LPC * D], f32, tag="xt")
            nc.sync.dma_start(xt_c[:], xt_d[:, lc * LPC * D : (lc + 1) * LPC * D])
            at_c = loadp.tile([128, LPC * C], bf16, tag="at")
            nc.sync.dma_start(at_c[:], at_d[:, lc * LPC * C : (lc + 1) * LPC * C])
            if lc == 0:
                nc.sync.dma_start(kn_all[:, D:], kn_d[:, D:])
            for li in range(LPC):
                _sel(lc * LPC + li, li, kt_c, xt_c, at_c)

        # ---- phase 2: streaming output ----
        for l in range(L):
            p_sb = ppool.tile([P_N, NF], bf16, tag="p")
            nc.gpsimd.dma_start(p_sb[:], p_d[l])
            ob = obuf.tile([BS, NF], bf16, tag="ob")
            for n in range(12):
                po = ps_o.tile([BS, 512], f32, tag="po")
                nc.tensor.matmul(
                    po[:],
                    wt_all[:, l * BS : (l + 1) * BS],
                    p_sb[:, n * 512 : (n + 1) * 512],
                    start=True,
                    stop=True,
                )
                if n % 2 == 0:
                    nc.scalar.copy(ob[:, n * 512 : (n + 1) * 512], po[:])
                else:
                    nc.vector.tensor_copy(ob[:, n * 512 : (n + 1) * 512], po[:])
            nc.scalar.dma_start(o_d[l], ob[:])

    nc.compile()
    _CACHE["nc"] = nc
    return nc


def _pack_inputs(x_query, K_all, A_all, P_all):
    import ml_dtypes

    bf = ml_dtypes.bfloat16
    x = np.asarray(x_query, dtype=np.float32)
    K = np.asarray(K_all, dtype=np.float32)
    A = np.asarray(A_all, dtype=np.float32)
    P = np.asarray(P_all, dtype=np.float32)

    # x^T per core: [128dd, (l, j, b)]
    xt = np.ascontiguousarray(
        x.reshape(N_CORES, BS, L, NCH, 128).transpose(0, 4, 2, 3, 1).reshape(
            N_CORES, 128, L * D
        )
    )
    # K^T / A^T: [128dd, (l, j, p)]
    kt = np.ascontiguousarray(
        K.reshape(L, P_N, NCH, 128).transpose(3, 0, 2, 1).reshape(128, L * C)
    )
    at = np.ascontiguousarray(
        A.reshape(L, P_N, NCH, 128).transpose(3, 0, 2, 1).reshape(128, L * C)
    ).astype(bf)
    # K natural: [p, (l, d)]
    kn = np.ascontiguousarray(K.transpose(1, 0, 2).reshape(P_N, L * D))
    pp = np.ascontiguousarray(P.reshape(L, P_N, NF)).astype(bf)
    return xt, kt, at, kn, pp


def _run(x_query, K_all, A_all, P_all, trace=False, tmpdir=None):
    from concourse.bass_utils import run_bass_kernel_spmd

    xt, kt, at, kn, pp = _pack_inputs(x_query, K_all, A_all, P_all)
    nc = _build_nc()
    in_maps = [
        {"x": xt[c], "kt": kt, "at": at, "kn": kn, "p": pp} for c in range(N_CORES)
    ]
    br = run_bass_kernel_spmd(
        nc, in_maps, list(range(N_CORES)), trace=trace, tmpdir=tmpdir
    )
    out = np.stack([np.asarray(r["o"]) for r in br.results], axis=0)  # [8, L, BS, NF]
    out = out.astype(np.float32).transpose(1, 0, 2, 3).reshape(L, B, LP, D)
    return out, br


def kernel(x_query, K_all, A_all, P_all):
    out, _ = _run(x_query, K_all, A_all, P_all)
    return out
